# revision 31
# baseline (speedup 1.0000x reference)
"""Trainium2 Bass kernel for nn_ModelNew_17411797418162.

Computation (per (b,s) sample):
  mixed = h_res @ x            # [4,4] @ [4,1024]
  out   = mixed * h_out[None,:] + h_post[:,None] * x

Sharding: pure data parallel over the leading batch dim B=8 -> 1 batch/core.

Per-core design (memory-bound). The correctness gate is rel_err < 2e-2,
so the whole datapath runs in fp16 (measured pipeline rel err ~7e-4),
halving HBM traffic vs fp32: per core x 16 MiB + out 16 MiB + h_out
4 MiB streamed per pass (+2 MiB block-diag h_res preloaded once)
-> ~106 us HBM roofline at 358 GB/s/core. Measured ~119 us/pass
(DMA-only ablation floor ~112 us), vs the 270 us fp32 baseline.

- Flatten (s, stream) -> rows: x/out [8192, 1024] fp16. x/out/h_out are
  pre-swizzled host-side into the SBUF tile layout (layout-only) so
  every DMA descriptor is a 16 KiB contiguous per-partition chunk;
  loads/stores are 2 MiB per DMA, with x loads alternating between the
  SP-HWDGE and SWDGE queues, stores on the ACT-HWDGE queue.
- Algebra: (h_res @ x) * h_out == h_res @ (x * h_out), so h_out is
  folded into x BEFORE the GEMM, and the h_post term is folded into the
  same PSUM accumulation as a diag(h_post) matmul:
    ps = W_blockdiag @ (x * ho4) + diag(h_post) @ x
  leaving a single PSUM->SBUF evacuation per tile (ACT) instead of a
  mul+add epilogue.
- ho4 (h_out replicated x4 onto the stream rows) is produced on the PE
  by a constant 0/1 [32->128] replication matmul into PSUM; the DVE
  multiplies x by it straight out of PSUM (1x mode, but saves an extra
  ACT evacuation pass).
- One PSUM tile per [128,1024] output tile serves BOTH the broadcast
  and the GEMM accumulation (the W matmul start=True overwrites it
  after the DVE consumed it - the WAR hazard coincides with the RAW
  dependency), so a single 4-deep PSUM pool fills all 8 banks and every
  cross-engine handoff has 2+ tiles of slack. A 1-tile-deep software
  pipeline (bcast t+1 emitted before GEMM t) keeps the PE off the DVE's
  latency.
- diag(h_post) [128,128] fp16 tiles are prebuilt once outside the loop
  by ACT scaling a constant identity (per-partition scale), avoiding a
  2 MiB DMA of pre-built diagonal matrices.
- Engine budget per core per pass (64 tiles): DMA 36 MiB ~112 us
  (bottleneck), PE 6 N=512 matmuls/tile ~85 us, DVE 1 TT/tile ~80 us,
  ACT 1 evacuation/tile ~86 us.
"""
import numpy as np

import concourse.bacc as bacc
import concourse.tile as tile
import concourse.mybir as mybir
from concourse.bass_utils import run_bass_kernel_spmd

B, S, N, D = 8, 2048, 4, 1024
NCORES = 8
ROWS = S * N              # 8192 flattened rows per core
NSB = 16                  # super-blocks per core (512 rows each)
SUBS = 4                  # sub-blocks (32 samples each) per super-block
GRP = 2                   # superblocks per x/out DMA (GRP MiB chunks)
GSUB = GRP * SUBS
NGRP = NSB // GRP
F32 = mybir.dt.float32
FP16 = mybir.dt.float16

_cache = {}


def build_program(iters: int = 1, mode: str = "full"):
    """Build the SPMD Bass program (one core's view). Cached per (iters, mode).

    mode: "full" = real kernel; ablations for bottleneck isolation:
      "dma"   = loads + store only (wrong output values)
      "nope"  = no matmuls: xh mul + evac only (wrong values)
    """
    if (iters, mode) in _cache:
        return _cache[(iters, mode)]

    nc = bacc.Bacc("TRN2", target_bir_lowering=False, debug=False)
    # x/out are pre-swizzled host-side to the SBUF tile layout so DMA
    # descriptors are 16 KiB contiguous per partition:
    #   x[g, p, 8192] with x[g, p, 1024*kk + d] = x_flat[1024g + 128kk + p, d]
    x = nc.dram_tensor("x", [NGRP, 128, GSUB * D], FP16,
                       kind="ExternalInput")
    w = nc.dram_tensor("w", [64, 128, 128], FP16, kind="ExternalInput")
    # ho swizzled per group: ho[g, p, 1024*h + d] = h_out[256g + 128h + p, d]
    ho = nc.dram_tensor("ho", [NGRP, 128, GRP * D], FP16,
                        kind="ExternalInput")
    e4 = nc.dram_tensor("e4", [128, 128], FP16, kind="ExternalInput")
    ident = nc.dram_tensor("ident", [128, 128], FP16, kind="ExternalInput")
    hp = nc.dram_tensor("hp", [128, 64], F32, kind="ExternalInput")
    out = nc.dram_tensor("out", [NGRP, 128, GSUB * D], FP16,
                         kind="ExternalOutput")

    with tile.TileContext(nc) as tc:
        with (
            tc.tile_pool(name="const", bufs=1) as cpool,
            tc.tile_pool(name="big", bufs=3) as bpool,
            tc.tile_pool(name="mid", bufs=3) as mpool,
            tc.tile_pool(name="pp_o", bufs=4, space="PSUM") as ppo,
        ):
            hp_all = cpool.tile([128, 64], F32)
            nc.gpsimd.dma_start(hp_all[:], hp.ap())
            e4_t = cpool.tile([128, 128], FP16)
            nc.gpsimd.dma_start(e4_t[:], e4.ap())
            id_t = cpool.tile([128, 128], FP16)
            nc.gpsimd.dma_start(id_t[:], ident.ap())
            # resident block-diag weights: w_all[r, (b, c)] = w[b, r, c]
            w_all = cpool.tile([128, 64 * 128], FP16)
            nc.gpsimd.dma_start(
                w_all[:].rearrange("r (b c) -> r b c", b=64),
                w.ap().rearrange("b r c -> r b c"))
            # prebuild all diag(h_post) blocks once: hpd_all[:, 128b:] =
            # diag(hp[:, b]) via ACT per-partition scale of the identity
            hpd_all = cpool.tile([128, 64 * 128], FP16)
            for blk in range(64):
                nc.scalar.activation(
                    hpd_all[:, 128 * blk:128 * (blk + 1)], id_t[:],
                    mybir.ActivationFunctionType.Copy,
                    scale=hp_all[:, blk:blk + 1])

            def load_grp(g):
                x_t = bpool.tile([128, GSUB * D], FP16, tag="x")
                eng = nc.sync if g % 2 == 0 else nc.gpsimd
                eng.dma_start(x_t[:], x.ap()[g])
                ho_t = mpool.tile([128, GRP * D], FP16, tag="ho")
                nc.gpsimd.dma_start(ho_t[:], ho.ap()[g])
                out_g = bpool.tile([128, GSUB * D], FP16, tag="out_g")
                return x_t, ho_t, out_g

            def store_grp(g, src_g):
                nc.scalar.dma_start(out.ap()[g], src_g[:])

            def bcast(ctx, t):
                # broadcast h_out onto stream rows via 0/1 matmul (PE).
                # The SAME full-width PSUM tile is reused by compute():
                # the DVE consumes the broadcast, then the W matmul
                # (start=True) overwrites it in place. One 4-deep PSUM
                # pool (8 banks) gives every cross-engine handoff two+
                # tiles of slack.
                x_t, ho_t, out_g = ctx
                kk, k = t % GSUB, t % SUBS
                h = (t // SUBS) % GRP
                ps = ppo.tile([128, D], F32, tag="ps")
                for c in range(2):
                    nc.tensor.matmul(
                        ps[:, 512 * c:512 * (c + 1)],
                        e4_t[32 * k:32 * (k + 1), :],
                        ho_t[32 * k:32 * (k + 1),
                             D * h + 512 * c:D * h + 512 * (c + 1)],
                        start=True, stop=True,
                        tile_position=(32 * k, 0))
                if mode == "novec":
                    return ps, x_t[:, D * kk:D * (kk + 1)]
                # xh = x * ho4 (DVE, one PSUM operand)
                xh = mpool.tile([128, D], FP16, tag="xh")
                nc.vector.tensor_mul(
                    xh[:], x_t[:, D * kk:D * (kk + 1)], ps[:])
                return ps, xh

            def compute(ctx, t, ps, xh):
                # ps = W @ xh + diag(hp) @ x  (PSUM accumulate, in-place
                # over the broadcast), then one full-width ACT evacuation
                x_t, ho_t, out_g = ctx
                kk, blk = t % GSUB, t
                lhsW = w_all[:, 128 * blk:128 * (blk + 1)]
                for c in range(2):
                    nc.tensor.matmul(
                        ps[:, 512 * c:512 * (c + 1)], lhsW,
                        xh[:, 512 * c:512 * (c + 1)],
                        start=True, stop=False)
                lhsH = hpd_all[:, 128 * blk:128 * (blk + 1)]
                for c in range(2):
                    nc.tensor.matmul(
                        ps[:, 512 * c:512 * (c + 1)], lhsH,
                        x_t[:, D * kk + 512 * c:D * kk + 512 * (c + 1)],
                        start=False, stop=True)
                nc.scalar.copy(out_g[:, D * kk:D * (kk + 1)], ps[:])

            def body():
                if mode == "dma":
                    for g in range(NSB // GRP):
                        x_t, ho_t, out_g = load_grp(g)
                        store_grp(g, x_t)
                    return
                if mode == "nope":
                    for g in range(NSB // GRP):
                        x_t, ho_t, out_g = load_grp(g)
                        for kk in range(GSUB):
                            xh = mpool.tile([128, D], FP16, tag="xh")
                            nc.vector.tensor_mul(
                                xh[:], x_t[:, D * kk:D * (kk + 1)],
                                x_t[:, D * kk:D * (kk + 1)])
                            nc.scalar.copy(out_g[:, D * kk:D * (kk + 1)],
                                           xh[:])
                        store_grp(g, out_g)
                    return
                # Software pipeline, 1 tile deep: the PE runs tile t+1's
                # broadcast while the DVE product for tile t is still in
                # flight, so the PE never stalls on the DVE.
                DEPTH = 1
                pipe = []
                for t in range(NSB * SUBS):
                    if t % GSUB == 0:
                        ctx = load_grp(t // GSUB)
                    ps, xh = bcast(ctx, t)
                    pipe.append((ctx, t, ps, xh))
                    if len(pipe) > DEPTH:
                        (pctx, pt, pps, pxh) = pipe.pop(0)
                        compute(pctx, pt, pps, pxh)
                        if pt % GSUB == GSUB - 1:
                            store_grp(pt // GSUB, pctx[2])
                for (pctx, pt, pps, pxh) in pipe:
                    compute(pctx, pt, pps, pxh)
                    if pt % GSUB == GSUB - 1:
                        store_grp(pt // GSUB, pctx[2])

            if iters == 1:
                body()
            else:
                with tc.For_i(0, iters, 1):
                    body()

    nc.compile()
    _cache[(iters, mode)] = nc
    return nc


def make_in_maps(x, h_res, h_out, h_post):
    """Split full inputs into per-core input maps (host-side, layout only)."""
    x = np.ascontiguousarray(x, dtype=np.float32)
    h_res = np.ascontiguousarray(h_res, dtype=np.float32)
    h_out = np.ascontiguousarray(h_out, dtype=np.float32)
    h_post = np.ascontiguousarray(h_post, dtype=np.float32)

    # stream-replication matrix: e4[q, 4*(q%32)+i] = 1
    e4 = np.zeros((128, 128), np.float16)
    q = np.arange(128)
    for i in range(4):
        e4[q, 4 * (q % 32) + i] = 1.0
    ident = np.eye(128, dtype=np.float16)

    in_maps = []
    for c in range(NCORES):
        xc = (x[c].reshape(ROWS, D).astype(np.float16)
              .reshape(NGRP, GSUB, 128, D).transpose(0, 2, 1, 3)
              .reshape(NGRP, 128, GSUB * D))
        xc = np.ascontiguousarray(xc)
        # Block-diagonal weights: W[b, 4p+j, 4p+i] = h_res[c, 32b+p, i, j]
        hr = h_res[c].reshape(64, 32, 4, 4)            # [b, p, i, j]
        Wb = np.zeros((64, 32, 4, 32, 4), np.float16)  # [b, (p,j), (p,i)]
        idx = np.arange(32)
        # advanced indexing: result axes (idx-bcast, b, j, i)
        Wb[:, idx, :, idx, :] = hr.transpose(1, 0, 3, 2).astype(np.float16)
        Wc = Wb.reshape(64, 128, 128)
        hpc = np.ascontiguousarray(
            h_post[c].reshape(64, 128).T)              # hp[p, b] = flat[128b+p]
        hoc = (h_out[c].astype(np.float16)
               .reshape(NGRP, GRP, 128, D).transpose(0, 2, 1, 3)
               .reshape(NGRP, 128, GRP * D))
        hoc = np.ascontiguousarray(hoc)
        in_maps.append({"x": xc, "w": Wc, "hp": hpc, "e4": e4,
                        "ident": ident, "ho": hoc})
    return in_maps


def kernel(x, h_res, h_out, h_post):
    nc = build_program(iters=1)
    in_maps = make_in_maps(x, h_res, h_out, h_post)
    res = run_bass_kernel_spmd(nc, in_maps, list(range(NCORES)))
    outs = []
    for c in range(NCORES):
        o = (res.results[c]["out"].reshape(NGRP, 128, GSUB, D)
             .transpose(0, 2, 1, 3).reshape(S, N, D))
        outs.append(o.astype(np.float32))
    return np.stack(outs)
